# revision 3
# baseline (speedup 1.0000x reference)
"""Trainium2 Bass kernel for DigitConvolutionalModel (conv3x3 -> FC512 -> FC10).

Math: the 3x3 valid conv is linear, so  y_flat = x @ C  with C [784, 676]
holding conv_w values in a banded structure.  Then
    logits = relu(x @ (C @ W1) + b1) @ W2 + b2
The fold W1_eff = C @ W1 is computed on device (banded matmul over only
the nonzero blocks), then the big [2048, 784] @ [784, 512] matmul per
core, relu, and the [*, 512] @ [512, 10] head.  Data-parallel across 8
cores on the batch dim.

v3 schedule: weights ride the two HW DMA rings in fold-consumption order
as fine-grained tiles (cmb halves + one tile per W1 m-chunk) so the fold
can start as soon as its first pieces land (~10.5us) instead of waiting
for everything; the fold is interleaved with the first batch-superblock's
L1 groups so weight-arrival jitter never idles the PE; x rides the same
rings after the weights (same-ring FIFO replaces dummy gating); the two
late-needed wide x tiles go to the slow SWDGE ring; logits leave per
superblock so the final DMA (which gates the fixed ~9us teardown)
completes right after the last L2 piece.
"""

import numpy as np
import ml_dtypes

B = 16384
IMG = 28
K = 3
OUT = IMG - K + 1  # 26
M26 = OUT * OUT  # 676
Q = IMG * IMG  # 784
HID = 512
NCLS = 10

NCORES = 8
BL = B // NCORES  # 2048 rows per core
QT = 112  # q-tile height (partition dim), 7 tiles
NQT = Q // QT  # 7
SB = 512  # batch superblock (matmul N)
NSB = BL // SB  # 4
NHT = HID // 128  # 4
NMC = (M26 + 127) // 128  # 6 m-chunks
NWARM = 8  # dummy matmuls riding out the PE HAM ramp + weight DMA

TRACE = False  # set by test harness to capture an NTFF profile
_CACHE = {}

_BF16 = ml_dtypes.bfloat16


def _band_blocks():
    """Static nonzero block pattern of C^T [676, 784] against (mc, qt) tiling."""
    Cs = np.zeros((Q, M26), dtype=bool)
    ii, jj = np.meshgrid(np.arange(OUT), np.arange(OUT), indexing="ij")
    m = (OUT * ii + jj).ravel()
    for di in range(K):
        for dj in range(K):
            q = ((ii + di) * IMG + (jj + dj)).ravel()
            Cs[q, m] = True
    CT = Cs.T  # [676, 784]
    blocks = []
    for t in range(NQT):
        mcs = []
        for mc in range(NMC):
            rows = min(128, M26 - 128 * mc)
            if CT[128 * mc : 128 * mc + rows, QT * t : QT * (t + 1)].any():
                mcs.append(mc)
        blocks.append(mcs)
    return blocks


_BLOCKS = _band_blocks()
_PAIRS = [(t, mc) for t in range(NQT) for mc in _BLOCKS[t]]
NP_ = len(_PAIRS)  # 14

# cmb pairs split between the two HW rings in fold order
_CM_SPLIT = [(0, 7), (7, NP_)]  # sync: pairs 0-6 (t0..t3), scalar: 7-13


def _build():
    import concourse.bacc as bacc
    import concourse.mybir as mybir
    import concourse.tile as tile

    f32 = mybir.dt.float32
    bf16 = mybir.dt.bfloat16
    AF = mybir.ActivationFunctionType

    nc = bacc.Bacc("TRN2", target_bir_lowering=False, debug=False)

    xt_d = nc.dram_tensor("xt", [Q, BL], bf16, kind="ExternalInput")
    cm_d = [
        nc.dram_tensor(f"cm{r}", [128, (hi - lo) * QT], bf16, kind="ExternalInput")
        for r, (lo, hi) in enumerate(_CM_SPLIT)
    ]
    w1_d = [
        nc.dram_tensor(f"w1m{mc}", [128, HID], bf16, kind="ExternalInput")
        for mc in range(NMC)
    ]
    b1_d = nc.dram_tensor("b1l", [128, NHT], f32, kind="ExternalInput")
    w2_d = nc.dram_tensor("w2l", [128, NHT * NCLS], bf16, kind="ExternalInput")
    b2_d = nc.dram_tensor("b2l", [NCLS, 1], f32, kind="ExternalInput")
    out_d = nc.dram_tensor("out", [NCLS, BL], f32, kind="ExternalOutput")

    pair_loc = {}
    for r, (lo, hi) in enumerate(_CM_SPLIT):
        for p in range(lo, hi):
            pair_loc[p] = (r, p - lo)

    with tile.TileContext(nc) as tc:
        with (
            tc.tile_pool(name="weights", bufs=1) as wp,
            tc.tile_pool(name="xin", bufs=1) as xp,
            tc.tile_pool(name="hid", bufs=1) as hp,
            tc.tile_pool(name="lgts", bufs=1) as lp,
            tc.tile_pool(name="psF", bufs=2, space="PSUM") as psF,
            tc.tile_pool(name="ps1", bufs=1, space="PSUM") as ps1p,
            tc.tile_pool(name="ps2", bufs=1, space="PSUM") as ps2p,
        ):
            # ---- PE warmup: scratch memset on gpsimd (free earliest after
            # the entry barrier) so the first matmul fires ~7.1us and the
            # ~5us half-rate PE power ramp finishes as early as possible.
            scratch = wp.tile([128, HID], bf16, tag="scratch")
            nc.gpsimd.memset(scratch[:], 0.0)
            warm = psF.tile([128, HID], f32, tag="ps")
            for i in range(NWARM):
                nc.tensor.matmul(
                    warm[:],
                    lhsT=scratch[:, :128],
                    rhs=scratch[:],
                    start=True,
                    stop=True,
                )

            # ---- weight + x DMAs on the two HW rings, in consumption
            # order.  Same-ring FIFO ordering guarantees weights-then-x
            # without dummy gating.  Tiny constants + the first two x tiles
            # lead so relu/L1 never wait on them.
            xsm, xw = {}, [None] * NQT

            b1 = wp.tile([128, NHT], f32, tag="b1")
            nc.sync.dma_start(out=b1[:], in_=b1_d[:, :])
            w2 = wp.tile([128, NHT * NCLS], bf16, tag="w2")
            nc.scalar.dma_start(out=w2[:], in_=w2_d[:, :])
            b2 = wp.tile([NCLS, 1], f32, tag="b2")
            nc.scalar.dma_start(out=b2[:], in_=b2_d[:, :])

            def xload(s, t, eng):
                xx = xp.tile([QT, SB], bf16, tag=f"x{s}_{t}")
                eng.dma_start(
                    out=xx[:],
                    in_=xt_d[QT * t : QT * (t + 1), SB * s : SB * (s + 1)],
                )
                xsm[(s, t)] = xx

            xload(0, 0, nc.sync)
            xload(0, 1, nc.scalar)

            cmb = []
            for r, (lo, hi) in enumerate(_CM_SPLIT):
                t_ = wp.tile([128, (hi - lo) * QT], bf16, tag=f"cmb{r}")
                (nc.sync if r == 0 else nc.scalar).dma_start(
                    out=t_[:], in_=cm_d[r][:, :]
                )
                cmb.append(t_)
            w1p = []
            for mc in range(NMC):
                t_ = wp.tile([128, HID], bf16, tag=f"w1m{mc}")
                (nc.sync if mc < 3 else nc.scalar).dma_start(
                    out=t_[:], in_=w1_d[mc][:, :]
                )
                w1p.append(t_)

            # remaining x: s=0,1 small tiles alternating rings; s=2,3 as
            # wide pair tiles, with the two latest-needed on the SWDGE ring
            for t in range(2, NQT):
                xload(0, t, nc.sync if t % 2 == 0 else nc.scalar)
            for t in range(NQT):
                xload(1, t, nc.sync if t % 2 == 1 else nc.scalar)
            for t in range(NQT):
                xx = xp.tile([QT, 2 * SB], bf16, tag=f"xw_{t}")
                eng = (
                    nc.gpsimd
                    if t >= 5
                    else (nc.sync if t % 2 == 0 else nc.scalar)
                )
                eng.dma_start(
                    out=xx[:], in_=xt_d[QT * t : QT * (t + 1), 2 * SB : BL]
                )
                xw[t] = xx

            def xslice(s, t):
                if s < 2:
                    return xsm[(s, t)][:]
                return xw[t][:, SB * (s - 2) : SB * (s - 1)]

            # ---- fold: W1_eff[q, h] = sum_m C^T[m, q] * W1[m, h] ----
            # Each fold block rotates through psF's two banks; its copy-out
            # frees the bank ~2 blocks before reuse.
            pair_idx = {pair: i for i, pair in enumerate(_PAIRS)}
            w1eff = []

            def fold_block(t):
                ps = psF.tile([QT, HID], f32, tag="ps", name=f"foldps_{t}")
                mcs = _BLOCKS[t]
                for j, mc in enumerate(mcs):
                    rows = min(128, M26 - 128 * mc)
                    p = pair_idx[(t, mc)]
                    pr, pslot = pair_loc[p]
                    nc.tensor.matmul(
                        ps[:],
                        lhsT=cmb[pr][:rows, QT * pslot : QT * (pslot + 1)],
                        rhs=w1p[mc][:rows, :],
                        start=(j == 0),
                        stop=(j == len(mcs) - 1),
                    )
                we = wp.tile([QT, HID], bf16, tag=f"we{t}", name=f"we{t}")
                half = HID // 2
                nc.vector.tensor_copy(we[:, :half], ps[:, :half])
                nc.scalar.activation(we[:, half:], ps[:, half:], AF.Copy)
                w1eff.append(we)

            hs_all = {}
            lg = lp.tile([NCLS, BL], f32, tag="lg")

            def l1_group(s, ht, ps1s):
                for t in range(NQT):
                    nc.tensor.matmul(
                        ps1s[ht][:],
                        lhsT=w1eff[t][:, 128 * ht : 128 * (ht + 1)],
                        rhs=xslice(s, t),
                        start=(t == 0),
                        stop=(t == NQT - 1),
                    )

            def relu(s, ht, ps1s):
                h = hp.tile([128, SB], bf16, tag=f"h{s}_{ht}", name=f"h{s}_{ht}")
                if s == NSB - 1:
                    half = SB // 2
                    nc.scalar.activation(
                        h[:, :half],
                        ps1s[ht][:, :half],
                        AF.Relu,
                        bias=b1[:, ht : ht + 1],
                        scale=1.0,
                    )
                    nc.vector.tensor_scalar(
                        h[:, half:],
                        ps1s[ht][:, half:],
                        b1[:, ht : ht + 1],
                        0.0,
                        mybir.AluOpType.add,
                        mybir.AluOpType.max,
                    )
                else:
                    nc.scalar.activation(
                        h[:],
                        ps1s[ht][:],
                        AF.Relu,
                        bias=b1[:, ht : ht + 1],
                        scale=1.0,
                    )
                hs_all[(s, ht)] = h

            def alloc_ps1():
                return [
                    ps1p.tile([128, SB], f32, tag=f"ps1_{ht}", name=f"ps1_{ht}")
                    for ht in range(NHT)
                ]

            def l1_block(s):
                ps1s = alloc_ps1()
                for ht in range(NHT):
                    l1_group(s, ht, ps1s)
                    relu(s, ht, ps1s)

            def l2_block(s):
                ps2 = ps2p.tile([NCLS, SB], f32, tag="ps2a", name=f"ps2_{s}")
                for ht in range(NHT):
                    nc.tensor.matmul(
                        ps2[:],
                        lhsT=w2[:, NCLS * ht : NCLS * (ht + 1)],
                        rhs=hs_all[(s, ht)][:],
                        start=(ht == 0),
                        stop=(ht == NHT - 1),
                    )
                half = SB // 2
                lo = SB * s
                nc.vector.tensor_scalar(
                    lg[:, lo : lo + half],
                    ps2[:, :half],
                    b2[:, 0:1],
                    None,
                    mybir.AluOpType.add,
                )
                nc.scalar.activation(
                    lg[:, lo + half : lo + SB],
                    ps2[:, half:],
                    AF.Identity,
                    bias=b2[:, 0:1],
                    scale=1.0,
                )
                eng = nc.sync if s % 2 == 0 else nc.scalar
                eng.dma_start(out=out_d[:, lo : lo + SB], in_=lg[:, lo : lo + SB])

            def l2_last():
                # s=3 in two half-N pieces: shorter closing chain, halves'
                # bias+DMA on independent engine pairs
                s = NSB - 1
                half = SB // 2
                lo = SB * s
                psa = ps2p.tile([NCLS, half], f32, tag="ps2a", name="ps2_3a")
                psb = ps2p.tile([NCLS, half], f32, tag="ps2b", name="ps2_3b")
                for ht in range(NHT):
                    nc.tensor.matmul(
                        psa[:],
                        lhsT=w2[:, NCLS * ht : NCLS * (ht + 1)],
                        rhs=hs_all[(s, ht)][:, :half],
                        start=(ht == 0),
                        stop=(ht == NHT - 1),
                    )
                nc.vector.tensor_scalar(
                    lg[:, lo : lo + half],
                    psa[:],
                    b2[:, 0:1],
                    None,
                    mybir.AluOpType.add,
                )
                nc.sync.dma_start(
                    out=out_d[:, lo : lo + half], in_=lg[:, lo : lo + half]
                )
                for ht in range(NHT):
                    nc.tensor.matmul(
                        psb[:],
                        lhsT=w2[:, NCLS * ht : NCLS * (ht + 1)],
                        rhs=hs_all[(s, ht)][:, half:],
                        start=(ht == 0),
                        stop=(ht == NHT - 1),
                    )
                nc.scalar.activation(
                    lg[:, lo + half : lo + SB],
                    psb[:],
                    AF.Identity,
                    bias=b2[:, 0:1],
                    scale=1.0,
                )
                nc.scalar.dma_start(
                    out=out_d[:, lo + half : lo + SB],
                    in_=lg[:, lo + half : lo + SB],
                )

            # ---- PE stream: fold interleaved with the first superblock's
            # L1 groups (s=0 is t-outer: group t feeds all 4 ht banks), so
            # the PE always has ready work while weight pieces trickle in.
            ps1s0 = alloc_ps1()
            for t in range(NQT):
                fold_block(t)
                for ht in range(NHT):
                    nc.tensor.matmul(
                        ps1s0[ht][:],
                        lhsT=w1eff[t][:, 128 * ht : 128 * (ht + 1)],
                        rhs=xslice(0, t),
                        start=(t == 0),
                        stop=(t == NQT - 1),
                    )
            for ht in range(NHT):
                relu(0, ht, ps1s0)
            l1_block(1)
            l2_block(0)
            l1_block(2)
            l2_block(1)
            l1_block(3)
            l2_block(2)
            l2_last()

    nc.compile()
    return nc


def _get_nc():
    if "nc" not in _CACHE:
        _CACHE["nc"] = _build()
    return _CACHE["nc"]


def kernel(x, conv_w, W1, b1, W2, b2):
    from concourse.bass_utils import run_bass_kernel_spmd

    nc = _get_nc()

    # C [784, 676]: y_flat = x @ C  (banded placement of conv_w values)
    C = np.zeros((Q, M26), dtype=np.float32)
    ii, jj = np.meshgrid(np.arange(OUT), np.arange(OUT), indexing="ij")
    m = (OUT * ii + jj).ravel()
    cw = np.asarray(conv_w, dtype=np.float32)
    for di in range(K):
        for dj in range(K):
            q = ((ii + di) * IMG + (jj + dj)).ravel()
            C[q, m] = cw[di, dj]
    CT = C.T  # [676, 784]
    cm_pieces = []
    for lo, hi in _CM_SPLIT:
        piece = np.zeros((128, (hi - lo) * QT), dtype=np.float32)
        for k, p in enumerate(range(lo, hi)):
            t, mc = _PAIRS[p]
            rows = min(128, M26 - 128 * mc)
            piece[:rows, QT * k : QT * (k + 1)] = CT[
                128 * mc : 128 * mc + rows, QT * t : QT * (t + 1)
            ]
        cm_pieces.append(piece.astype(_BF16))

    w1f = np.asarray(W1, np.float32)
    w1_pieces = []
    for mc in range(NMC):
        piece = np.zeros((128, HID), dtype=np.float32)
        rows = min(128, M26 - 128 * mc)
        piece[:rows] = w1f[128 * mc : 128 * mc + rows, :]
        w1_pieces.append(piece.astype(_BF16))

    b1l = np.ascontiguousarray(
        np.asarray(b1, np.float32).reshape(NHT, 128).T
    )  # [128, 4]
    w2l = np.ascontiguousarray(
        np.asarray(W2, np.float32)
        .reshape(NHT, 128, NCLS)
        .transpose(1, 0, 2)
        .reshape(128, NHT * NCLS)
    ).astype(_BF16)
    b2l = np.asarray(b2, np.float32).reshape(NCLS, 1)

    xf = np.asarray(x, np.float32)
    in_maps = []
    for c in range(NCORES):
        xt = np.ascontiguousarray(xf[c * BL : (c + 1) * BL].T).astype(_BF16)
        im = {"xt": xt, "b1l": b1l, "w2l": w2l, "b2l": b2l}
        for r in range(len(_CM_SPLIT)):
            im[f"cm{r}"] = cm_pieces[r]
        for mc in range(NMC):
            im[f"w1m{mc}"] = w1_pieces[mc]
        in_maps.append(im)

    kwargs = {}
    if TRACE:
        import profhook  # noqa: F401  (installs the NTFF hook shim)
        import tempfile

        kwargs = {"trace": True, "tmpdir": tempfile.mkdtemp(prefix="ntff_")}
    res = run_bass_kernel_spmd(nc, in_maps, core_ids=list(range(NCORES)), **kwargs)
    if TRACE:
        _CACHE["last_results"] = res

    out = np.concatenate(
        [np.ascontiguousarray(res.results[c]["out"].T) for c in range(NCORES)], axis=0
    ).astype(np.float32)
    return out


# revision 10
# speedup vs baseline: 1.1476x; 1.1476x over previous
"""Trainium2 Bass kernel for DigitConvolutionalModel (conv3x3 -> FC512 -> FC10).

Math: the 3x3 valid conv is linear, so  y_flat = x @ C  with C [784, 676]
holding conv_w values in a banded structure.  Then
    logits = relu(x @ (C @ W1) + b1) @ W2 + b2
The fold W1_eff = C @ W1 is computed on device (banded matmul over only
the nonzero blocks), then the big [2048, 784] @ [784, 512] matmul per
core, relu, and the [*, 512] @ [512, 10] head.  Data-parallel across 8
cores on the batch dim.

v4 schedule: DMA triggers BLOCK the issuing engine's instruction queue
(only 4 outstanding transfers per ring), so all bulk x traffic rides the
compute-free sync + gpsimd rings; scalar carries only 6 early weight
triggers and is free for relus by ~11us; vector carries none.  Weights
go in fold-consumption order as fine-grained tiles (cmb halves + one
tile per W1 m-chunk) so the fold starts ~10us; the fold is
software-pipelined with the first superblock's L1 groups (fold_t+1 runs
between fold_t and L1_t to cover the PSUM->SBUF copy latency); x(0,0..2)
lead on gpsimd so L1 never waits on x; logits leave per superblock so
the final DMA (which gates the fixed ~9us teardown) completes right
after the last L2 piece.
"""

import numpy as np
import ml_dtypes

B = 16384
IMG = 28
K = 3
OUT = IMG - K + 1  # 26
M26 = OUT * OUT  # 676
Q = IMG * IMG  # 784
HID = 512
NCLS = 10

NCORES = 8
BL = B // NCORES  # 2048 rows per core
QT = 112  # q-tile height (partition dim), 7 tiles
NQT = Q // QT  # 7
SB = 512  # batch superblock (matmul N)
NSB = BL // SB  # 4
NHT = HID // 128  # 4
NMC = (M26 + 127) // 128  # 6 m-chunks
NWARM = 7  # dummy matmuls riding out the PE HAM ramp + weight DMA

TRACE = False  # set by test harness to capture an NTFF profile
_CACHE = {}

_BF16 = ml_dtypes.bfloat16


def _band_blocks():
    """Static nonzero block pattern of C^T [676, 784] against (mc, qt) tiling."""
    Cs = np.zeros((Q, M26), dtype=bool)
    ii, jj = np.meshgrid(np.arange(OUT), np.arange(OUT), indexing="ij")
    m = (OUT * ii + jj).ravel()
    for di in range(K):
        for dj in range(K):
            q = ((ii + di) * IMG + (jj + dj)).ravel()
            Cs[q, m] = True
    CT = Cs.T  # [676, 784]
    blocks = []
    for t in range(NQT):
        mcs = []
        for mc in range(NMC):
            rows = min(128, M26 - 128 * mc)
            if CT[128 * mc : 128 * mc + rows, QT * t : QT * (t + 1)].any():
                mcs.append(mc)
        blocks.append(mcs)
    return blocks


_BLOCKS = _band_blocks()
_PAIRS = [(t, mc) for t in range(NQT) for mc in _BLOCKS[t]]
NP_ = len(_PAIRS)  # 14

# cmb pairs split between the two HW rings in fold order
_CM_SPLIT = [(0, 7), (7, NP_)]  # sync: pairs 0-6 (t0..t3), scalar: 7-13


def _build():
    import concourse.bacc as bacc
    import concourse.mybir as mybir
    import concourse.tile as tile

    f32 = mybir.dt.float32
    bf16 = mybir.dt.bfloat16
    AF = mybir.ActivationFunctionType

    nc = bacc.Bacc("TRN2", target_bir_lowering=False, debug=False)

    xt_d = nc.dram_tensor("xt", [Q, BL], bf16, kind="ExternalInput")
    cm_d = [
        nc.dram_tensor(f"cm{r}", [128, (hi - lo) * QT], bf16, kind="ExternalInput")
        for r, (lo, hi) in enumerate(_CM_SPLIT)
    ]
    w1_d = [
        nc.dram_tensor(f"w1m{mc}", [128, HID], bf16, kind="ExternalInput")
        for mc in range(NMC)
    ]
    b1_d = nc.dram_tensor("b1l", [128, NHT], f32, kind="ExternalInput")
    w2_d = nc.dram_tensor("w2l", [128, NHT * NCLS], bf16, kind="ExternalInput")
    b2_d = nc.dram_tensor("b2l", [NCLS, 1], f32, kind="ExternalInput")
    out_d = nc.dram_tensor("out", [NCLS, BL], f32, kind="ExternalOutput")

    pair_loc = {}
    for r, (lo, hi) in enumerate(_CM_SPLIT):
        for p in range(lo, hi):
            pair_loc[p] = (r, p - lo)

    with tile.TileContext(nc) as tc:
        with (
            tc.tile_pool(name="weights", bufs=1) as wp,
            tc.tile_pool(name="xin", bufs=1) as xp,
            tc.tile_pool(name="hid", bufs=1) as hp,
            tc.tile_pool(name="lgts", bufs=1) as lp,
            tc.tile_pool(name="psF", bufs=2, space="PSUM") as psF,
            tc.tile_pool(name="ps1", bufs=1, space="PSUM") as ps1p,
            tc.tile_pool(name="ps2", bufs=1, space="PSUM") as ps2p,
        ):
            # ---- PE warmup: scratch memset on gpsimd (free earliest after
            # the entry barrier) so the first matmul fires ~7.1us and the
            # ~5us half-rate PE power ramp finishes as early as possible.
            scratch = wp.tile([128, HID], bf16, tag="scratch")
            nc.gpsimd.memset(scratch[:], 0.0)
            warm = psF.tile([128, HID], f32, tag="ps")
            for i in range(NWARM):
                nc.tensor.matmul(
                    warm[:],
                    lhsT=scratch[:, :128],
                    rhs=scratch[:],
                    start=True,
                    stop=True,
                )

            # ---- DMA plan.  sync: b1 + its weight share + the bulk of x.
            # gpsimd (SWDGE): the first three x(0,*) tiles (needed earliest
            # by the interleaved fold/L1 stream) + the late wide x tiles.
            # scalar: 6 early weight triggers only, free for relus by ~11us.
            xsm, xw = {}, [None] * NQT

            def xload(s, t, eng):
                xx = xp.tile([QT, SB], bf16, tag=f"x{s}_{t}")
                eng.dma_start(
                    out=xx[:],
                    in_=xt_d[QT * t : QT * (t + 1), SB * s : SB * (s + 1)],
                )
                xsm[(s, t)] = xx

            def xwload(t, eng):
                xx = xp.tile([QT, 2 * SB], bf16, tag=f"xw_{t}")
                eng.dma_start(
                    out=xx[:], in_=xt_d[QT * t : QT * (t + 1), 2 * SB : BL]
                )
                xw[t] = xx

            # gpsimd ring
            for t in range(3):
                xload(0, t, nc.gpsimd)

            # sync ring: weights first, then x in consumption order
            b1 = wp.tile([128, NHT], f32, tag="b1")
            nc.sync.dma_start(out=b1[:], in_=b1_d[:, :])
            cmb = [None, None]
            cmb[0] = wp.tile(
                [128, _CM_SPLIT[0][1] * QT], bf16, tag="cmb0", name="cmb0"
            )
            nc.sync.dma_start(out=cmb[0][:], in_=cm_d[0][:, :])
            w1p = [None] * NMC
            for mc in range(3):
                w1p[mc] = wp.tile([128, HID], bf16, tag=f"w1m{mc}", name=f"w1m{mc}")
                nc.sync.dma_start(out=w1p[mc][:], in_=w1_d[mc][:, :])

            # scalar ring: its weight share only
            w2 = wp.tile([128, NHT * NCLS], bf16, tag="w2")
            nc.scalar.dma_start(out=w2[:], in_=w2_d[:, :])
            b2 = wp.tile([NCLS, 1], f32, tag="b2")
            nc.scalar.dma_start(out=b2[:], in_=b2_d[:, :])
            cmb[1] = wp.tile(
                [128, (_CM_SPLIT[1][1] - _CM_SPLIT[1][0]) * QT],
                bf16,
                tag="cmb1",
                name="cmb1",
            )
            nc.scalar.dma_start(out=cmb[1][:], in_=cm_d[1][:, :])
            for mc in range(3, NMC):
                w1p[mc] = wp.tile([128, HID], bf16, tag=f"w1m{mc}", name=f"w1m{mc}")
                nc.scalar.dma_start(out=w1p[mc][:], in_=w1_d[mc][:, :])

            # rest of x on sync (consumption order); late wide tiles on gpsimd
            for t in range(3, NQT):
                xload(0, t, nc.sync)
            for t in range(NQT):
                xload(1, t, nc.sync)
            for t in range(3):
                xwload(t, nc.sync)
            for t in range(3, NQT):
                xwload(t, nc.gpsimd)

            def xslice(s, t):
                if s < 2:
                    return xsm[(s, t)][:]
                return xw[t][:, SB * (s - 2) : SB * (s - 1)]

            # ---- fold: W1_eff[q, h] = sum_m C^T[m, q] * W1[m, h] ----
            # Each fold block rotates through psF's two banks; its copy-out
            # frees the bank ~2 blocks before reuse.
            pair_idx = {pair: i for i, pair in enumerate(_PAIRS)}
            w1eff = []

            def fold_block(t):
                ps = psF.tile([QT, HID], f32, tag="ps", name=f"foldps_{t}")
                mcs = _BLOCKS[t]
                for j, mc in enumerate(mcs):
                    rows = min(128, M26 - 128 * mc)
                    p = pair_idx[(t, mc)]
                    pr, pslot = pair_loc[p]
                    nc.tensor.matmul(
                        ps[:],
                        lhsT=cmb[pr][:rows, QT * pslot : QT * (pslot + 1)],
                        rhs=w1p[mc][:rows, :],
                        start=(j == 0),
                        stop=(j == len(mcs) - 1),
                    )
                # copy-out on vector only: scalar's queue is stuck behind its
                # weight DMA triggers until ~11us
                we = wp.tile([QT, HID], bf16, tag=f"we{t}", name=f"we{t}")
                nc.vector.tensor_copy(we[:], ps[:])
                w1eff.append(we)

            hs_all = {}
            lg = lp.tile([NCLS, BL], f32, tag="lg")

            def l1_group(s, ht, ps1s):
                for t in range(NQT):
                    nc.tensor.matmul(
                        ps1s[ht][:],
                        lhsT=w1eff[t][:, 128 * ht : 128 * (ht + 1)],
                        rhs=xslice(s, t),
                        start=(t == 0),
                        stop=(t == NQT - 1),
                    )

            def relu(s, ht, ps1s):
                h = hp.tile([128, SB], bf16, tag=f"h{s}_{ht}", name=f"h{s}_{ht}")
                if s == NSB - 1:
                    half = SB // 2
                    nc.scalar.activation(
                        h[:, :half],
                        ps1s[ht][:, :half],
                        AF.Relu,
                        bias=b1[:, ht : ht + 1],
                        scale=1.0,
                    )
                    nc.vector.tensor_scalar(
                        h[:, half:],
                        ps1s[ht][:, half:],
                        b1[:, ht : ht + 1],
                        0.0,
                        mybir.AluOpType.add,
                        mybir.AluOpType.max,
                    )
                else:
                    nc.scalar.activation(
                        h[:],
                        ps1s[ht][:],
                        AF.Relu,
                        bias=b1[:, ht : ht + 1],
                        scale=1.0,
                    )
                hs_all[(s, ht)] = h

            def alloc_ps1():
                return [
                    ps1p.tile([128, SB], f32, tag=f"ps1_{ht}", name=f"ps1_{ht}")
                    for ht in range(NHT)
                ]

            def l1_block(s):
                ps1s = alloc_ps1()
                for ht in range(NHT):
                    l1_group(s, ht, ps1s)
                    relu(s, ht, ps1s)

            def l2_block(s):
                ps2 = ps2p.tile([NCLS, SB], f32, tag="ps2a", name=f"ps2_{s}")
                for ht in range(NHT):
                    nc.tensor.matmul(
                        ps2[:],
                        lhsT=w2[:, NCLS * ht : NCLS * (ht + 1)],
                        rhs=hs_all[(s, ht)][:],
                        start=(ht == 0),
                        stop=(ht == NHT - 1),
                    )
                half = SB // 2
                lo = SB * s
                nc.vector.tensor_scalar(
                    lg[:, lo : lo + half],
                    ps2[:, :half],
                    b2[:, 0:1],
                    None,
                    mybir.AluOpType.add,
                )
                nc.scalar.activation(
                    lg[:, lo + half : lo + SB],
                    ps2[:, half:],
                    AF.Identity,
                    bias=b2[:, 0:1],
                    scale=1.0,
                )
                eng = nc.sync if s % 2 == 0 else nc.scalar
                eng.dma_start(out=out_d[:, lo : lo + SB], in_=lg[:, lo : lo + SB])

            def l2_last():
                # s=3 in two half-N pieces: shorter closing chain, halves'
                # bias+DMA on independent engine pairs
                s = NSB - 1
                half = SB // 2
                lo = SB * s
                psa = ps2p.tile([NCLS, half], f32, tag="ps2a", name="ps2_3a")
                psb = ps2p.tile([NCLS, half], f32, tag="ps2b", name="ps2_3b")
                for ht in range(NHT):
                    nc.tensor.matmul(
                        psa[:],
                        lhsT=w2[:, NCLS * ht : NCLS * (ht + 1)],
                        rhs=hs_all[(s, ht)][:, :half],
                        start=(ht == 0),
                        stop=(ht == NHT - 1),
                    )
                nc.vector.tensor_scalar(
                    lg[:, lo : lo + half],
                    psa[:],
                    b2[:, 0:1],
                    None,
                    mybir.AluOpType.add,
                )
                nc.sync.dma_start(
                    out=out_d[:, lo : lo + half], in_=lg[:, lo : lo + half]
                )
                for ht in range(NHT):
                    nc.tensor.matmul(
                        psb[:],
                        lhsT=w2[:, NCLS * ht : NCLS * (ht + 1)],
                        rhs=hs_all[(s, ht)][:, half:],
                        start=(ht == 0),
                        stop=(ht == NHT - 1),
                    )
                nc.scalar.activation(
                    lg[:, lo + half : lo + SB],
                    psb[:],
                    AF.Identity,
                    bias=b2[:, 0:1],
                    scale=1.0,
                )
                nc.scalar.dma_start(
                    out=out_d[:, lo + half : lo + SB],
                    in_=lg[:, lo + half : lo + SB],
                )

            # ---- PE stream: fold software-pipelined with the first
            # superblock's L1 groups (s=0 is t-outer: group t feeds all 4
            # ht banks).  fold_{t+1} runs between fold_t and L1_t so the
            # we_t PSUM->SBUF copy is never on the PE's critical path.
            ps1s0 = alloc_ps1()

            def l1s0_group(t):
                for ht in range(NHT):
                    nc.tensor.matmul(
                        ps1s0[ht][:],
                        lhsT=w1eff[t][:, 128 * ht : 128 * (ht + 1)],
                        rhs=xslice(0, t),
                        start=(t == 0),
                        stop=(t == NQT - 1),
                    )

            fold_block(0)
            for t in range(1, NQT):
                fold_block(t)
                l1s0_group(t - 1)
            l1s0_group(NQT - 1)
            for ht in range(NHT):
                relu(0, ht, ps1s0)
            l1_block(1)
            l2_block(0)
            l1_block(2)
            l2_block(1)
            l1_block(3)
            l2_block(2)
            l2_last()

    nc.compile()
    return nc


def _get_nc():
    if "nc" not in _CACHE:
        _CACHE["nc"] = _build()
    return _CACHE["nc"]


def kernel(x, conv_w, W1, b1, W2, b2):
    from concourse.bass_utils import run_bass_kernel_spmd

    nc = _get_nc()

    # C [784, 676]: y_flat = x @ C  (banded placement of conv_w values)
    C = np.zeros((Q, M26), dtype=np.float32)
    ii, jj = np.meshgrid(np.arange(OUT), np.arange(OUT), indexing="ij")
    m = (OUT * ii + jj).ravel()
    cw = np.asarray(conv_w, dtype=np.float32)
    for di in range(K):
        for dj in range(K):
            q = ((ii + di) * IMG + (jj + dj)).ravel()
            C[q, m] = cw[di, dj]
    CT = C.T  # [676, 784]
    cm_pieces = []
    for lo, hi in _CM_SPLIT:
        piece = np.zeros((128, (hi - lo) * QT), dtype=np.float32)
        for k, p in enumerate(range(lo, hi)):
            t, mc = _PAIRS[p]
            rows = min(128, M26 - 128 * mc)
            piece[:rows, QT * k : QT * (k + 1)] = CT[
                128 * mc : 128 * mc + rows, QT * t : QT * (t + 1)
            ]
        cm_pieces.append(piece.astype(_BF16))

    w1f = np.asarray(W1, np.float32)
    w1_pieces = []
    for mc in range(NMC):
        piece = np.zeros((128, HID), dtype=np.float32)
        rows = min(128, M26 - 128 * mc)
        piece[:rows] = w1f[128 * mc : 128 * mc + rows, :]
        w1_pieces.append(piece.astype(_BF16))

    b1l = np.ascontiguousarray(
        np.asarray(b1, np.float32).reshape(NHT, 128).T
    )  # [128, 4]
    w2l = np.ascontiguousarray(
        np.asarray(W2, np.float32)
        .reshape(NHT, 128, NCLS)
        .transpose(1, 0, 2)
        .reshape(128, NHT * NCLS)
    ).astype(_BF16)
    b2l = np.asarray(b2, np.float32).reshape(NCLS, 1)

    xf = np.asarray(x, np.float32)
    in_maps = []
    for c in range(NCORES):
        xt = np.ascontiguousarray(xf[c * BL : (c + 1) * BL].T).astype(_BF16)
        im = {"xt": xt, "b1l": b1l, "w2l": w2l, "b2l": b2l}
        for r in range(len(_CM_SPLIT)):
            im[f"cm{r}"] = cm_pieces[r]
        for mc in range(NMC):
            im[f"w1m{mc}"] = w1_pieces[mc]
        in_maps.append(im)

    kwargs = {}
    if TRACE:
        import profhook  # noqa: F401  (installs the NTFF hook shim)
        import tempfile

        kwargs = {"trace": True, "tmpdir": tempfile.mkdtemp(prefix="ntff_")}
    res = run_bass_kernel_spmd(nc, in_maps, core_ids=list(range(NCORES)), **kwargs)
    if TRACE:
        _CACHE["last_results"] = res

    out = np.concatenate(
        [np.ascontiguousarray(res.results[c]["out"].T) for c in range(NCORES)], axis=0
    ).astype(np.float32)
    return out
